# revision 5
# baseline (speedup 1.0000x reference)
"""Trainium2 Bass kernel for sparse (sliding-window residual) attention.

Problem (hardcoded): B=1, N=2048, D=1024, H=16, HD=64, window +-128.
reference:
  q = rope(x@Wq+bq), k = rope(x@Wk+bk), v = x@Wv+bv   (16 heads of 64)
  s = q k^T / 8
  full_p = softmax(s | keymask), win_p = softmax(s | keymask & window)
  out = (full_p v reshaped) @ Wo + bo, rows zeroed where ~mask
  window_residual = full_p v - win_p v
returns (out [1,2048,1024] f32, window_residual [1,16,2048,64] f32)

Sharding: head-parallel across 8 cores (2 heads each). Residual outputs
need no communication. For the output projection, each core's normalized
attn^T [128(inner-slice), 2048] is exchanged with AllToAll so core c ends
up with ALL heads for query window [256c, 256c+256) and computes that row
slice of out.

Layout: scores are computed transposed (sT[k,q], head-dim contraction on
partitions), exp'd WITHOUT max subtraction (scores here are bounded,
|s|/8 < ~4, and masked keys get a -1e9 per-partition ACT bias which
underflows exp to exactly 0). The windowed softmax reuses the same
unnormalized p restricted to the diagonal band: for 128-wide q/k tiles and
window +-128 that is tile (qt) unmasked plus tiles (qt-1, qt+1) under two
static triangular masks. Softmax denominators come free as a ones-column
appended to v in the attention matmul; 1/l is applied as a per-partition
ACT scale in [q, d] layout.
"""

import sys

sys.path.insert(0, "/opt/trn_rl_repo")

import numpy as np
import ml_dtypes

import concourse.bass as bass
import concourse.mybir as mybir
import concourse.tile as tile
from concourse import bacc
from concourse.bass_utils import run_bass_kernel_spmd

BF16 = ml_dtypes.bfloat16
N_CORES = 8
B, N, D, H, HD = 1, 2048, 1024, 16, 64
HPC = H // N_CORES          # heads per core = 2
DSL = HPC * HD              # inner-dim slice per core = 128
NROWS = N // N_CORES        # output rows per core = 256
NT = N // 128               # 16 n/k tiles
DT = D // 128               # 8 D tiles
QC = 1024                   # q-chunk width for scores/exp
NQC = N // QC
SCALE = 1.0 / 8.0           # 1/sqrt(HD)

F32 = mybir.dt.float32
F32R = mybir.dt.float32r
BF = mybir.dt.bfloat16

LAST_RESULT = None
_CACHE = {}
ts, ds = bass.ts, bass.ds


def _build():
    nc = bacc.Bacc("TRN2", target_bir_lowering=False, debug=False,
                   num_devices=N_CORES)

    def din(name, shape, dt):
        return nc.dram_tensor(name, shape, dt, kind="ExternalInput").ap()

    xtb = din("xtb", [D, N], BF)            # x^T, bf16
    wqb = din("wqb", [D, DSL], BF)          # Wq[:, c*128:(c+1)*128]
    wkb = din("wkb", [D, DSL], BF)
    wvb = din("wvb", [D, DSL], BF)
    bq = din("bq", [DSL, 1], F32)
    bk = din("bk", [DSL, 1], F32)
    bv = din("bv", [DSL, 1], F32)
    cosw = din("cosw", [DSL, N], F32)       # cos(freqs)^T tiled to 128 rows
    sinw = din("sinw", [DSL, N], F32)
    keyb = din("keyb", [128, NT], F32)      # key-mask bias, col kt = keys of tile kt
    permt = din("permt", [128, 128], F32R)   # rotate-half permutation (as lhsT)
    mlow = din("mlow", [128, 128], BF)      # 1 where pk >= fq
    mup = din("mup", [128, 128], BF)        # 1 where pk <= fq
    identb = din("identb", [128, 128], BF)  # identity for TensorE transposes
    wob = din("wob", [D, D], BF)            # full Wo
    bob = din("bob", [1, D], BF)            # bo row

    out_rows = nc.dram_tensor("out_rows", [NROWS, D], F32,
                              kind="ExternalOutput").ap()
    res = nc.dram_tensor("res", [HPC, N, HD], F32,
                         kind="ExternalOutput").ap()

    # AllToAll buffers: shard j (flat block j) = my attn^T for q-window j.
    a2a_in = nc.dram_tensor("a2a_in", [N_CORES, DSL, NROWS], BF,
                            kind="Internal").ap()
    a2a_out = nc.dram_tensor("a2a_out", [N_CORES, DSL, NROWS], BF,
                             kind="Internal").ap()

    with tile.TileContext(nc) as tc:
        with (
            tc.tile_pool(name="const", bufs=1) as cpool,
            tc.tile_pool(name="work", bufs=1) as wpool,
            tc.tile_pool(name="small", bufs=2) as spool,
            tc.tile_pool(name="ps", bufs=2, space="PSUM") as pspool,
            tc.tile_pool(name="pa", bufs=2, space="PSUM") as papool,
        ):
            # ---- constants / weights to SBUF ----
            cos_sb = cpool.tile([128, N], F32, tag="cos")
            sin_sb = cpool.tile([128, N], F32, tag="sin")
            nc.sync.dma_start(cos_sb[:], cosw)
            nc.sync.dma_start(sin_sb[:], sinw)
            keyb_sb = cpool.tile([128, NT], F32, tag="keyb")
            nc.sync.dma_start(keyb_sb[:], keyb)
            perm_sb = cpool.tile([128, 128], F32R, tag="perm")
            nc.sync.dma_start(perm_sb[:], permt)
            mlow_sb = cpool.tile([128, 128], BF, tag="mlow")
            nc.sync.dma_start(mlow_sb[:], mlow)
            mup_sb = cpool.tile([128, 128], BF, tag="mup")
            nc.sync.dma_start(mup_sb[:], mup)
            id_sb = cpool.tile([128, 128], BF, tag="ident")
            nc.sync.dma_start(id_sb[:], identb)
            bqs = cpool.tile([128, 1], F32, tag="bq")
            nc.sync.dma_start(bqs[:], bq)
            bks = cpool.tile([128, 1], F32, tag="bk")
            nc.sync.dma_start(bks[:], bk)
            bvs = cpool.tile([128, 1], F32, tag="bv")
            nc.sync.dma_start(bvs[:], bv)
            bob_sb = cpool.tile([1, D], BF, tag="bob")
            nc.sync.dma_start(bob_sb[:], bob)
            ones_sb = cpool.tile([1, 128], BF, tag="ones")
            nc.vector.memset(ones_sb[:], 1.0)

            wq_sb = cpool.tile([128, DT, DSL], BF, tag="wq")
            wk_sb = cpool.tile([128, DT, DSL], BF, tag="wk")
            wv_sb = cpool.tile([128, DT, DSL], BF, tag="wv")
            for t in range(DT):
                nc.sync.dma_start(wq_sb[:, t, :], wqb[ts(t, 128), :])
                nc.sync.dma_start(wk_sb[:, t, :], wkb[ts(t, 128), :])
                nc.sync.dma_start(wv_sb[:, t, :], wvb[ts(t, 128), :])

            # x^T tiles ("big" tag: 4MB slots shared with p-slabs and Wo)
            xt_sb = wpool.tile([128, DT, N], BF, tag="big", bufs=3)
            for t in range(DT):
                nc.sync.dma_start(xt_sb[:, t, :], xtb[ts(t, 128), :])

            # ---- projections: qT/kT/vT [128(inner-slice), N] ----
            qraw = wpool.tile([128, N], F32R, tag="qraw")
            kraw = wpool.tile([128, N], F32R, tag="kraw")
            vtb = wpool.tile([128, N], BF, tag="vtb")
            for (w_sb, b_sb, dst) in ((wq_sb, bqs, qraw), (wk_sb, bks, kraw),
                                      (wv_sb, bvs, vtb)):
                for ch in range(N // 512):
                    ps = pspool.tile([128, 512], F32, tag="st")
                    for t in range(DT):
                        nc.tensor.matmul(
                            ps[:], w_sb[:, t, :], xt_sb[:, t, ts(ch, 512)],
                            start=(t == 0), stop=(t == DT - 1))
                    nc.vector.tensor_scalar_add(dst[:, ts(ch, 512)],
                                                ps[:], b_sb[:, 0:1])

            # ---- rope: qr/kr = raw*cos + rotate_half(raw)*sin ----
            qr = wpool.tile([128, N], F32R, tag="qr")
            kr = wpool.tile([128, N], F32R, tag="kr")
            for (src, dst) in ((qraw, qr), (kraw, kr)):
                for ch in range(N // 512):
                    sl = ts(ch, 512)
                    ph = pspool.tile([128, 512], F32, tag="st")
                    nc.tensor.matmul(ph[:], perm_sb[:],
                                     src[:, sl],
                                     start=True, stop=True)
                    tmp = spool.tile([128, 512], F32, tag="ropetmp")
                    nc.vector.tensor_mul(tmp[:], ph[:], sin_sb[:, sl])
                    nc.vector.tensor_mul(dst[:, sl], src[:, sl], cos_sb[:, sl])
                    nc.vector.tensor_add(dst[:, sl], dst[:, sl], tmp[:])

            # ---- v -> [n, d] layout with ones columns ----
            # per n-tile: [:, 0:64] head0, col 64 ones, [65:129] head1, col 129 ones
            v_sb = wpool.tile([128, NT, 130], BF, tag="vsb")
            for t in range(NT):
                pt = papool.tile([128, 128], BF, tag="tr")
                nc.tensor.transpose(pt[:], vtb[:, ts(t, 128)], id_sb[:])
                nc.vector.tensor_copy(v_sb[:, t, 0:64], pt[:, 0:64])
                nc.vector.tensor_copy(v_sb[:, t, 65:129], pt[:, 64:128])
                nc.vector.memset(v_sb[:, t, 64:65], 1.0)
                nc.vector.memset(v_sb[:, t, 129:130], 1.0)

            # ---- attention per head ----
            agt = wpool.tile([128, N], BF, tag="agt")  # normalized attn^T
            for h in range(HPC):
                hsl = slice(64 * h, 64 * h + 64)
                vcol = slice(65 * h, 65 * h + 65)
                for qc in range(NQC):
                    # scores^T -> exp -> p-slab [k = 128 x kt, q = QC]
                    slab = wpool.tile([128, NT, QC], BF, tag="big", bufs=3)
                    for kt in range(NT):
                        ps = pspool.tile([128, QC], F32, tag="st")
                        for sub in range(QC // 512):
                            nc.tensor.matmul(
                                ps[:, ts(sub, 512)],
                                kr[hsl, ts(kt, 128)],
                                qr[hsl, ds(qc * QC + sub * 512, 512)],
                                start=True, stop=True)
                        nc.scalar.activation(slab[:, kt, :], ps[:],
                                             mybir.ActivationFunctionType.Exp,
                                             bias=keyb_sb[:, kt:kt + 1],
                                             scale=SCALE)
                    # attention for the q-tiles of this chunk
                    for qtl in range(QC // 128):
                        qt = qc * (QC // 128) + qtl
                        qsl = ts(qtl, 128)
                        aug = papool.tile([128, 130], F32, tag="aug")
                        for kt in range(NT):
                            nc.tensor.matmul(aug[:, 0:65],
                                             slab[:, kt, qsl],
                                             v_sb[:, kt, vcol],
                                             start=(kt == 0), stop=(kt == NT - 1))
                        # windowed: tiles qt-1 (pk>=fq), qt (full), qt+1 (pk<=fq)
                        wk_tiles = []
                        if qt > 0:
                            wk_tiles.append((qt - 1, mlow_sb))
                        wk_tiles.append((qt, None))
                        if qt < NT - 1:
                            wk_tiles.append((qt + 1, mup_sb))
                        for i, (kt, msk) in enumerate(wk_tiles):
                            if msk is None:
                                lhs = slab[:, kt, qsl]
                            else:
                                mskd = spool.tile([128, 128], BF, tag="mskd")
                                nc.vector.tensor_mul(mskd[:], slab[:, kt, qsl],
                                                     msk[:])
                                lhs = mskd[:]
                            nc.tensor.matmul(aug[:, 65:130], lhs,
                                             v_sb[:, kt, vcol],
                                             start=(i == 0),
                                             stop=(i == len(wk_tiles) - 1))
                        # normalize + residual
                        rf = spool.tile([128, 1], F32, tag="rf")
                        rw = spool.tile([128, 1], F32, tag="rw")
                        nc.vector.reciprocal(rf[:], aug[:, 64:65])
                        nc.vector.reciprocal(rw[:], aug[:, 129:130])
                        tf = spool.tile([128, 64], F32, tag="tf")
                        tw = spool.tile([128, 64], F32, tag="tw")
                        nc.scalar.activation(tf[:], aug[:, 0:64],
                                             mybir.ActivationFunctionType.Copy,
                                             scale=rf[:, 0:1])
                        nc.scalar.activation(tw[:], aug[:, 65:129],
                                             mybir.ActivationFunctionType.Copy,
                                             scale=rw[:, 0:1])
                        rsb = spool.tile([128, 64], F32, tag="rsb")
                        nc.vector.tensor_sub(rsb[:], tf[:], tw[:])
                        nc.sync.dma_start(res[h, ts(qt, 128), :], rsb[:])
                        # bf16 normalized attn, transposed into AG payload
                        tb = spool.tile([128, 64], BF, tag="tb")
                        nc.vector.tensor_copy(tb[:], tf[:])
                        ptr = papool.tile([64, 128], BF, tag="tr")
                        nc.tensor.transpose(ptr[:], tb[:], id_sb[:])
                        nc.vector.tensor_copy(agt[hsl, ts(qt, 128)], ptr[:])

            # ---- AllToAll: shard j = my attn^T q-cols [256j, 256j+256) ----
            for j in range(N_CORES):
                nc.sync.dma_start(a2a_in[j, :, :], agt[:, ts(j, NROWS)])
            nc.gpsimd.collective_compute(
                "AllToAll", mybir.AluOpType.bypass,
                replica_groups=[list(range(N_CORES))],
                ins=[a2a_in], outs=[a2a_out])

            # ---- output projection for my NROWS rows ----
            wo_sb = wpool.tile([128, DT, D], BF, tag="big", bufs=3)
            for t in range(DT):
                nc.sync.dma_start(wo_sb[:, t, :], wob[ts(t, 128), :])
            g_sb = wpool.tile([128, DT, NROWS], BF, tag="gsb")
            for r in range(N_CORES):
                nc.sync.dma_start(g_sb[:, r, :], a2a_out[r, :, :])
            for nb in range(NROWS // 128):
                for dc in range(D // 512):
                    po = pspool.tile([128, 512], F32, tag="st")
                    for r in range(DT):
                        nc.tensor.matmul(po[:], g_sb[:, r, ts(nb, 128)],
                                         wo_sb[:, r, ts(dc, 512)],
                                         start=(r == 0), stop=False)
                    nc.tensor.matmul(po[:], ones_sb[0:1, :],
                                     bob_sb[0:1, ts(dc, 512)],
                                     start=False, stop=True)
                    osb = spool.tile([128, 512], F32, tag="osb")
                    nc.vector.tensor_copy(osb[:], po[:])
                    nc.sync.dma_start(out_rows[ts(nb, 128), ts(dc, 512)], osb[:])

    nc.compile()
    return nc


def _prep_inputs(x, mask, freqs, Wq, bq, Wk, bk, Wv, bv, Wo, bo):
    """Host-side sharding / layout prep. Returns per-core in_maps."""
    x2 = np.asarray(x, np.float32).reshape(N, D)
    xt = np.ascontiguousarray(x2.T).astype(BF16)            # [D, N]
    f = np.asarray(freqs, np.float32)                       # [N, HD]
    # reference applies rope to inner dims [0:rot] with rot = freqs width =
    # 64, i.e. ONLY head 0 (core 0, rows 0:64) is rotated; everything else
    # passes through: cos=1, sin=0.
    def rope_tables(c):
        cw = np.ones((DSL, N), np.float32)
        sw = np.zeros((DSL, N), np.float32)
        if c == 0:
            cw[0:HD] = np.cos(f).T
            sw[0:HD] = np.sin(f).T
        return np.ascontiguousarray(cw), np.ascontiguousarray(sw)
    m = np.asarray(mask).reshape(N)
    keyb = np.ascontiguousarray(
        np.where(m, np.float32(0.0), np.float32(-1e9)).reshape(NT, 128).T)
    permt = np.zeros((128, 128), np.float32)
    idx = np.arange(0, 128, 2)
    permt[idx + 1, idx] = -1.0                              # qh[2i] = -q[2i+1]
    permt[idx, idx + 1] = 1.0                               # qh[2i+1] = q[2i]
    pk = np.arange(128)[:, None]
    fq = np.arange(128)[None, :]
    mlow = (pk >= fq).astype(BF16)
    mup = (pk <= fq).astype(BF16)
    identb = np.eye(128, dtype=BF16)
    wo_b = np.ascontiguousarray(np.asarray(Wo, np.float32)).astype(BF16)
    bo_b = np.asarray(bo, np.float32).reshape(1, D).astype(BF16)

    common = dict(xtb=xt, keyb=keyb, permt=permt,
                  mlow=np.ascontiguousarray(mlow),
                  mup=np.ascontiguousarray(mup),
                  identb=np.ascontiguousarray(identb),
                  wob=wo_b, bob=np.ascontiguousarray(bo_b))
    in_maps = []
    for c in range(N_CORES):
        sl = slice(c * DSL, (c + 1) * DSL)
        cw, sw = rope_tables(c)
        in_maps.append(dict(
            common,
            cosw=cw, sinw=sw,
            wqb=np.ascontiguousarray(np.asarray(Wq, np.float32)[:, sl]).astype(BF16),
            wkb=np.ascontiguousarray(np.asarray(Wk, np.float32)[:, sl]).astype(BF16),
            wvb=np.ascontiguousarray(np.asarray(Wv, np.float32)[:, sl]).astype(BF16),
            bq=np.ascontiguousarray(np.asarray(bq, np.float32)[sl, None]),
            bk=np.ascontiguousarray(np.asarray(bk, np.float32)[sl, None]),
            bv=np.ascontiguousarray(np.asarray(bv, np.float32)[sl, None]),
        ))
    return in_maps


def kernel(**inputs):
    global LAST_RESULT
    if "nc" not in _CACHE:
        _CACHE["nc"] = _build()
    nc = _CACHE["nc"]
    in_maps = _prep_inputs(**inputs)
    r = run_bass_kernel_spmd(nc, in_maps, core_ids=list(range(N_CORES)))
    LAST_RESULT = r
    out = np.concatenate([r.results[c]["out_rows"] for c in range(N_CORES)],
                         axis=0)
    m = np.asarray(inputs["mask"]).reshape(N)
    out = np.where(m[:, None], out, np.float32(0.0))[None]          # [1, N, D]
    resid = np.concatenate([r.results[c]["res"] for c in range(N_CORES)],
                           axis=0)[None]                            # [1, H, N, HD]
    return (out, resid)


if __name__ == "__main__":
    _build()
    print("build+compile OK")
